# revision 15
# baseline (speedup 1.0000x reference)
"""BinaryConv2d (3x3, SAME, NHWC) on 8 trn2 NeuronCores.

Sharding: data-parallel over batch — 2 images per core; the tiny binarized
weight tensor is replicated. Per core, the two images are packed on the two
64-partition halves of SBUF so each 3x3-tap matmul pair (K=64 contraction =
C_in) runs concurrently on disjoint row-groups of the 128x128 PE array.

Each matmul covers exactly 2 output rows (N = 2*224 = 448) via a 2D rhs
access pattern over the zero-padded 226-wide plane, so the output stream is
dense — no garbage columns and 112 uniform slots per image.
"""

import sys

for _p in ("/opt/trn_rl_repo",):
    if _p not in sys.path:
        sys.path.insert(0, _p)

import ml_dtypes
import numpy as np

BF16 = ml_dtypes.bfloat16

N_CORES = 8
IMG_PER_CORE = 2
H = W_IMG = 224
C_IN, C_OUT = 64, 128
PR = PC = 226  # padded plane: 224 data rows/cols + 1 zero ring
PLANE = PR * PC  # 51076
NSLOT = 2 * W_IMG  # 448 outputs per matmul = 2 dense image rows
N_SLOTS = 112  # 112 * 448 = 50176 = 224*224, exact
NTAPS = 9
QOUT = H * W_IMG  # 50176 dense outputs per image

# slots per x-input chunk: first 2 slots ride in the hot transfer, then a
# supply-matched ramp (head DMA bandwidth ramps ~100->450 GB/s over ~10us)
CHUNKS = [2, 2, 4, 8, 16, 16, 16, 16, 16, 16]
XTILE_ROWS = 2 * max(CHUNKS) + 2  # 34 padded rows
XTILE_COLS = XTILE_ROWS * PC  # 7684
# slots per output stage: small tail stages so the final out-DMA is tiny
STAGES = [8] * 13 + [4, 2, 1, 1]
STAGE_Q = 8 * NSLOT  # 3584
N_WARM = 9
HOT_ROWS = 6  # x_pad rows 0-5: slots 0-1
HOT_COLS = 3 * 128 + HOT_ROWS * PC  # w taps 0-2, then x rows: 1740

_COMPILED = None
_LAST_RES = None


def _build():
    import concourse.mybir as mybir
    import concourse.tile as tile
    from concourse import bacc

    nc = bacc.Bacc(
        "TRN2", target_bir_lowering=False, debug=False, num_devices=N_CORES
    )
    x_d = nc.dram_tensor("x", [128, PLANE], mybir.dt.bfloat16, kind="ExternalInput")
    w_d = nc.dram_tensor(
        "w", [128, NTAPS * 128], mybir.dt.bfloat16, kind="ExternalInput"
    )
    hot_d = nc.dram_tensor(
        "hot", [128, HOT_COLS], mybir.dt.bfloat16, kind="ExternalInput"
    )
    b_d = nc.dram_tensor("b", [128, 1], mybir.dt.float32, kind="ExternalInput")
    o_d = nc.dram_tensor(
        "out", [128, IMG_PER_CORE * QOUT], mybir.dt.bfloat16, kind="ExternalOutput"
    )

    ident = mybir.ActivationFunctionType.Identity

    chunk_plan = {}
    s = 0
    for n in CHUNKS:
        chunk_plan[s] = n
        s += n
    assert s == N_SLOTS
    stage_plan = {}
    s = 0
    for n in STAGES:
        stage_plan[s] = n
        s += n
    assert s == N_SLOTS

    with tile.TileContext(nc) as tc:
        with (
            tc.tile_pool(name="const", bufs=1) as cpool,
            tc.tile_pool(name="xin", bufs=4) as xpool,
            tc.tile_pool(name="stage", bufs=3) as spool,
            tc.tile_pool(name="psum", bufs=3, space="PSUM") as ppool,
        ):
            # Each DMA has a ~2.4us fixed issue->semaphore latency, so the
            # whole slot-0 working set (w taps 0-2 + x_pad rows 0-3) ships as
            # ONE early "hot" transfer; taps 0-2 are served from this
            # resident tile for every slot. Taps 3-8 follow in a second DMA
            # that lands before the cold stream reaches tap 3.
            ht = cpool.tile([128, HOT_COLS], mybir.dt.bfloat16, tag="hot")
            nc.sync.dma_start(ht[:], hot_d[:])
            w_sb = cpool.tile([128, NTAPS * 128], mybir.dt.bfloat16, tag="w")
            nc.sync.dma_start(w_sb[:, 384:1152], w_d[:, 384:1152])
            b_sb = cpool.tile([128, 1], mybir.dt.float32, tag="b")
            xv0 = ht[:, 384 : 384 + HOT_ROWS * PC].rearrange("p (r w) -> p r w", w=PC)

            # One HAM activity window (~3.4us) of dummy cold matmuls on a
            # zeroed tile, sized to finish as the first x chunk lands: the
            # PE clock-gate releases before the real stream starts, so it
            # runs at 2.4GHz early in the real stream (results never read).
            warm_src = cpool.tile([128, NSLOT], mybir.dt.bfloat16, tag="warm")
            nc.gpsimd.memset(warm_src[:], 0.0)
            warm_ps = ppool.tile([128, 512], mybir.dt.float32, tag="pswarm", bufs=1)
            for i in range(N_WARM):
                nc.tensor.matmul(
                    warm_ps[:, :NSLOT],
                    lhsT=warm_src[:, 0:128],
                    rhs=warm_src[:, :],
                    start=(i == 0),
                    stop=(i == N_WARM - 1),
                )

            xv = None
            ca = 0
            st_a = st_b = None
            g0 = 0
            gext = 0
            stage_end = -1
            for s in range(N_SLOTS):
                if s in chunk_plan:
                    ca = s
                    if s == 0:
                        xv = xv0  # slot 0 reads the resident hot tile
                        nc.sync.dma_start(b_sb[:], b_d[:])
                    else:
                        n_c = chunk_plan[s]
                        ext = (2 * n_c + 2) * PC
                        xt = xpool.tile([128, XTILE_COLS], mybir.dt.bfloat16, tag="x")
                        nc.sync.dma_start(
                            xt[:, :ext], x_d[:, 2 * ca * PC : 2 * ca * PC + ext]
                        )
                        xv = xt[:, :ext].rearrange("p (r w) -> p r w", w=PC)

                if s in stage_plan:
                    g0 = s * NSLOT
                    gext = stage_plan[s] * NSLOT
                    stage_end = s + stage_plan[s] - 1
                    st_a = spool.tile([128, STAGE_Q], mybir.dt.bfloat16, tag="sa")
                    st_b = spool.tile([128, STAGE_Q], mybir.dt.bfloat16, tag="sb")

                psa = ppool.tile([128, 512], mybir.dt.float32, tag="psa")
                psb = ppool.tile([128, 512], mybir.dt.float32, tag="psb")

                for t in range(NTAPS):
                    dh, dw = divmod(t, 3)
                    r0 = 2 * (s - ca) + dh
                    first, last = t == 0, t == NTAPS - 1
                    wt = ht if t < 3 else w_sb
                    nc.tensor.matmul(
                        psa[:, :NSLOT],
                        lhsT=wt[0:64, t * 128 : (t + 1) * 128],
                        rhs=xv[0:64, r0 : r0 + 2, dw : dw + W_IMG],
                        start=first,
                        stop=last,
                    )
                    nc.tensor.matmul(
                        psb[:, :NSLOT],
                        lhsT=wt[64:128, t * 128 : (t + 1) * 128],
                        rhs=xv[64:128, r0 : r0 + 2, dw : dw + W_IMG],
                        start=first,
                        stop=last,
                    )

                so = s * NSLOT - g0
                nc.vector.tensor_scalar_add(
                    st_a[:, so : so + NSLOT], psa[:, :NSLOT], b_sb[:]
                )
                nc.scalar.activation(
                    st_b[:, so : so + NSLOT], psb[:, :NSLOT], ident, bias=b_sb[:]
                )

                if s == stage_end:
                    nc.sync.dma_start(o_d[:, g0 : g0 + gext], st_a[:, :gext])
                    nc.scalar.dma_start(
                        o_d[:, QOUT + g0 : QOUT + g0 + gext], st_b[:, :gext]
                    )

    nc.compile()
    return nc


def _get_nc():
    global _COMPILED
    if _COMPILED is None:
        _COMPILED = _build()
    return _COMPILED


def kernel(x: np.ndarray, W: np.ndarray, b: np.ndarray) -> np.ndarray:
    from concourse.bass_utils import run_bass_kernel_spmd

    nc = _get_nc()

    xb = np.asarray(x, dtype=np.float32).astype(BF16)
    X = np.zeros((N_CORES, IMG_PER_CORE, C_IN, PR, PC), BF16)
    X[:, :, :, 1 : H + 1, 1 : W_IMG + 1] = xb.reshape(
        N_CORES, IMG_PER_CORE, H, W_IMG, C_IN
    ).transpose(0, 1, 4, 2, 3)
    Xf = X.reshape(N_CORES, 128, PLANE)

    Wb = np.sign(np.asarray(W, dtype=np.float32)).astype(BF16).reshape(NTAPS, C_IN, C_OUT)
    wh = np.empty((2, C_IN, NTAPS, C_OUT), BF16)
    wh[:] = Wb.transpose(1, 0, 2)[None]
    wh = np.ascontiguousarray(wh.reshape(128, NTAPS * C_OUT))

    bh = np.ascontiguousarray(np.asarray(b, dtype=np.float32).reshape(128, 1))

    hot = [
        np.ascontiguousarray(
            np.concatenate([wh[:, : 3 * 128], Xf[c][:, : HOT_ROWS * PC]], axis=1)
        )
        for c in range(N_CORES)
    ]

    in_maps = [
        {"x": Xf[c], "w": wh, "b": bh, "hot": hot[c]} for c in range(N_CORES)
    ]
    res = run_bass_kernel_spmd(nc, in_maps, list(range(N_CORES)))
    global _LAST_RES
    _LAST_RES = res

    O = np.stack([res.results[c]["out"] for c in range(N_CORES)])
    O = O.reshape(N_CORES, C_OUT, IMG_PER_CORE, H, W_IMG)
    y = O.transpose(0, 2, 3, 4, 1).reshape(16, H, W_IMG, C_OUT)
    return np.ascontiguousarray(y).astype(np.float32)


# revision 20
# speedup vs baseline: 1.1979x; 1.1979x over previous
"""BinaryConv2d (3x3, SAME, NHWC) on 8 trn2 NeuronCores.

Sharding: data-parallel over batch — 2 images per core; the tiny binarized
weight tensor is replicated. Per core, the two images are packed on the two
64-partition halves of SBUF so each 3x3-tap matmul pair (K=64 contraction =
C_in) runs concurrently on disjoint row-groups of the 128x128 PE array.

Each matmul covers exactly 2 output rows (N = 2*224 = 448) via a 2D rhs
access pattern over the zero-padded 226-wide plane, so the output stream is
dense — no garbage columns and 112 uniform slots per image.
"""

import sys

for _p in ("/opt/trn_rl_repo",):
    if _p not in sys.path:
        sys.path.insert(0, _p)

import ml_dtypes
import numpy as np

BF16 = ml_dtypes.bfloat16

N_CORES = 8
IMG_PER_CORE = 2
H = W_IMG = 224
C_IN, C_OUT = 64, 128
PR = PC = 226  # padded plane: 224 data rows/cols + 1 zero ring
PLANE = PR * PC  # 51076
NSLOT = 2 * W_IMG  # 448 outputs per matmul = 2 dense image rows
N_SLOTS = 112  # 112 * 448 = 50176 = 224*224, exact
NTAPS = 9
QOUT = H * W_IMG  # 50176 dense outputs per image

# slots per x-input chunk: first 2 slots ride in the hot transfer, then a
# supply-matched ramp (head DMA bandwidth ramps ~100->450 GB/s over ~10us)
CHUNKS = [2, 2, 4, 8, 16, 16, 16, 16, 16, 16]
XTILE_ROWS = 2 * max(CHUNKS) + 2  # 34 padded rows
XTILE_COLS = XTILE_ROWS * PC  # 7684
# slots per output stage: small tail stages so the final out-DMA is tiny
STAGES = [8] * 13 + [4, 2, 1, 1]
STAGE_Q = 8 * NSLOT  # 3584
N_WARM = 9
HOT_ROWS = 6  # x_pad rows 0-5: slots 0-1
HOT_COLS = 3 * 128 + HOT_ROWS * PC  # w taps 0-2, then x rows: 1740

_COMPILED = None
_LAST_RES = None


def _build():
    import concourse.mybir as mybir
    import concourse.tile as tile
    from concourse import bacc

    nc = bacc.Bacc(
        "TRN2", target_bir_lowering=False, debug=False, num_devices=N_CORES
    )
    x_d = nc.dram_tensor("x", [128, PLANE], mybir.dt.bfloat16, kind="ExternalInput")
    w_d = nc.dram_tensor(
        "w", [128, NTAPS * 128], mybir.dt.bfloat16, kind="ExternalInput"
    )
    hot_d = nc.dram_tensor(
        "hot", [128, HOT_COLS], mybir.dt.bfloat16, kind="ExternalInput"
    )
    b_d = nc.dram_tensor("b", [128, 1], mybir.dt.float32, kind="ExternalInput")
    o_d = nc.dram_tensor(
        "out", [128, IMG_PER_CORE * QOUT], mybir.dt.bfloat16, kind="ExternalOutput"
    )

    ident = mybir.ActivationFunctionType.Identity

    chunk_plan = {}
    s = 0
    for n in CHUNKS:
        chunk_plan[s] = n
        s += n
    assert s == N_SLOTS
    stage_plan = {}
    s = 0
    for n in STAGES:
        stage_plan[s] = n
        s += n
    assert s == N_SLOTS

    with tile.TileContext(nc) as tc:
        with (
            tc.tile_pool(name="const", bufs=1) as cpool,
            tc.tile_pool(name="xin", bufs=4) as xpool,
            tc.tile_pool(name="stage", bufs=3) as spool,
            tc.tile_pool(name="psum", bufs=3, space="PSUM") as ppool,
        ):
            # Each DMA has a ~2.4us fixed issue->semaphore latency, so the
            # whole slot-0 working set (w taps 0-2 + x_pad rows 0-3) ships as
            # ONE early "hot" transfer; taps 0-2 are served from this
            # resident tile for every slot. Taps 3-8 follow in a second DMA
            # that lands before the cold stream reaches tap 3.
            ht = cpool.tile([128, HOT_COLS], mybir.dt.bfloat16, tag="hot")
            nc.sync.dma_start(ht[:], hot_d[:])
            w_sb = cpool.tile([128, NTAPS * 128], mybir.dt.bfloat16, tag="w")
            nc.sync.dma_start(w_sb[:, 384:1152], w_d[:, 384:1152])
            b_sb = cpool.tile([128, 1], mybir.dt.float32, tag="b")
            xv0 = ht[:, 384 : 384 + HOT_ROWS * PC].rearrange("p (r w) -> p r w", w=PC)

            # One HAM activity window (~3.4us) of dummy cold matmuls on a
            # zeroed tile, sized to finish as the first x chunk lands: the
            # PE clock-gate releases before the real stream starts, so it
            # runs at 2.4GHz early in the real stream (results never read).
            warm_src = cpool.tile([128, NSLOT], mybir.dt.bfloat16, tag="warm")
            nc.gpsimd.memset(warm_src[:], 0.0)
            warm_ps = ppool.tile([128, 512], mybir.dt.float32, tag="pswarm", bufs=1)
            for i in range(N_WARM):
                nc.tensor.matmul(
                    warm_ps[:, :NSLOT],
                    lhsT=warm_src[:, 0:128],
                    rhs=warm_src[:, :],
                    start=(i == 0),
                    stop=(i == N_WARM - 1),
                )

            xv = None
            ca = 0
            st_a = st_b = None
            g0 = 0
            gext = 0
            stage_end = -1
            for s in range(N_SLOTS):
                if s in chunk_plan:
                    ca = s
                    if s == 0:
                        xv = xv0  # slot 0 reads the resident hot tile
                        nc.sync.dma_start(b_sb[:], b_d[:])
                    else:
                        n_c = chunk_plan[s]
                        ext = (2 * n_c + 2) * PC
                        xt = xpool.tile([128, XTILE_COLS], mybir.dt.bfloat16, tag="x")
                        nc.sync.dma_start(
                            xt[:, :ext], x_d[:, 2 * ca * PC : 2 * ca * PC + ext]
                        )
                        xv = xt[:, :ext].rearrange("p (r w) -> p r w", w=PC)

                if s in stage_plan:
                    g0 = s * NSLOT
                    gext = stage_plan[s] * NSLOT
                    stage_end = s + stage_plan[s] - 1
                    st_a = spool.tile([128, STAGE_Q], mybir.dt.bfloat16, tag="sa")
                    st_b = spool.tile([128, STAGE_Q], mybir.dt.bfloat16, tag="sb")

                psa = ppool.tile([128, 512], mybir.dt.float32, tag="psa")
                psb = ppool.tile([128, 512], mybir.dt.float32, tag="psb")

                for t in range(NTAPS):
                    dh, dw = divmod(t, 3)
                    r0 = 2 * (s - ca) + dh
                    first, last = t == 0, t == NTAPS - 1
                    wt = ht if t < 3 else w_sb
                    nc.tensor.matmul(
                        psa[:, :NSLOT],
                        lhsT=wt[0:64, t * 128 : (t + 1) * 128],
                        rhs=xv[0:64, r0 : r0 + 2, dw : dw + W_IMG],
                        start=first,
                        stop=last,
                    )
                    nc.tensor.matmul(
                        psb[:, :NSLOT],
                        lhsT=wt[64:128, t * 128 : (t + 1) * 128],
                        rhs=xv[64:128, r0 : r0 + 2, dw : dw + W_IMG],
                        start=first,
                        stop=last,
                    )

                so = s * NSLOT - g0
                nc.vector.tensor_scalar_add(
                    st_a[:, so : so + NSLOT], psa[:, :NSLOT], b_sb[:]
                )
                nc.scalar.activation(
                    st_b[:, so : so + NSLOT], psb[:, :NSLOT], ident, bias=b_sb[:]
                )

                if s == stage_end:
                    nc.sync.dma_start(o_d[:, g0 : g0 + gext], st_a[:, :gext])
                    nc.scalar.dma_start(
                        o_d[:, QOUT + g0 : QOUT + g0 + gext], st_b[:, :gext]
                    )

    nc.compile()
    return nc


def _get_nc():
    global _COMPILED
    if _COMPILED is None:
        _COMPILED = _build()
    return _COMPILED


def kernel(x: np.ndarray, W: np.ndarray, b: np.ndarray) -> np.ndarray:
    from concourse.bass_utils import run_bass_kernel_spmd

    nc = _get_nc()

    xb = np.asarray(x, dtype=np.float32).astype(BF16)
    X = np.zeros((N_CORES, IMG_PER_CORE, C_IN, PR, PC), BF16)
    X[:, :, :, 1 : H + 1, 1 : W_IMG + 1] = xb.reshape(
        N_CORES, IMG_PER_CORE, H, W_IMG, C_IN
    ).transpose(0, 1, 4, 2, 3)
    Xf = X.reshape(N_CORES, 128, PLANE)

    Wb = np.sign(np.asarray(W, dtype=np.float32)).astype(BF16).reshape(NTAPS, C_IN, C_OUT)
    wh = np.empty((2, C_IN, NTAPS, C_OUT), BF16)
    wh[:] = Wb.transpose(1, 0, 2)[None]
    wh = np.ascontiguousarray(wh.reshape(128, NTAPS * C_OUT))

    bh = np.ascontiguousarray(np.asarray(b, dtype=np.float32).reshape(128, 1))

    hot = [
        np.ascontiguousarray(
            np.concatenate([wh[:, : 3 * 128], Xf[c][:, : HOT_ROWS * PC]], axis=1)
        )
        for c in range(N_CORES)
    ]

    in_maps = [
        {"x": Xf[c], "w": wh, "b": bh, "hot": hot[c]} for c in range(N_CORES)
    ]
    res = run_bass_kernel_spmd(nc, in_maps, list(range(N_CORES)))
    global _LAST_RES
    _LAST_RES = res

    O = np.stack([res.results[c]["out"] for c in range(N_CORES)])
    O = O.reshape(N_CORES, C_OUT, IMG_PER_CORE, H, W_IMG)
    y = O.transpose(0, 2, 3, 4, 1).reshape(16, H, W_IMG, C_OUT)
    return np.ascontiguousarray(y).astype(np.float32)


# revision 21
# speedup vs baseline: 1.2019x; 1.0033x over previous
"""BinaryConv2d (3x3, SAME, NHWC) on 8 trn2 NeuronCores.

Sharding: data-parallel over batch — 2 images per core; the tiny binarized
weight tensor is replicated. Per core, the two images are packed on the two
64-partition halves of SBUF so each 3x3-tap matmul pair (K=64 contraction =
C_in) runs concurrently on disjoint row-groups of the 128x128 PE array.

Each matmul covers exactly 2 output rows (N = 2*224 = 448) via a 2D rhs
access pattern over the zero-padded 226-wide plane, so the output stream is
dense — no garbage columns and 112 uniform slots per image.
"""

import sys

for _p in ("/opt/trn_rl_repo",):
    if _p not in sys.path:
        sys.path.insert(0, _p)

import ml_dtypes
import numpy as np

BF16 = ml_dtypes.bfloat16

N_CORES = 8
IMG_PER_CORE = 2
H = W_IMG = 224
C_IN, C_OUT = 64, 128
PR = PC = 226  # padded plane: 224 data rows/cols + 1 zero ring
PLANE = PR * PC  # 51076
NSLOT = 2 * W_IMG  # 448 outputs per matmul = 2 dense image rows
N_SLOTS = 112  # 112 * 448 = 50176 = 224*224, exact
NTAPS = 9
QOUT = H * W_IMG  # 50176 dense outputs per image

# slots per x-input chunk: first 2 slots ride in the hot transfer, then a
# supply-matched ramp (head DMA bandwidth ramps ~100->450 GB/s over ~10us)
CHUNKS = [2, 2, 4, 8, 16, 16, 16, 16, 16, 16]
XTILE_ROWS = 2 * max(CHUNKS) + 2  # 34 padded rows
XTILE_COLS = XTILE_ROWS * PC  # 7684
# slots per output stage: small tail stages so the final out-DMA is tiny
STAGES = [8] * 13 + [4, 2, 1, 1]
STAGE_Q = 8 * NSLOT  # 3584
N_WARM = 9
HOT_ROWS = 6  # x_pad rows 0-5: slots 0-1
HOT_COLS = 3 * 128 + HOT_ROWS * PC  # w taps 0-2, then x rows: 1740

_COMPILED = None
_LAST_RES = None


def _build():
    import concourse.mybir as mybir
    import concourse.tile as tile
    from concourse import bacc

    nc = bacc.Bacc(
        "TRN2", target_bir_lowering=False, debug=False, num_devices=N_CORES
    )
    x_d = nc.dram_tensor("x", [128, PLANE], mybir.dt.bfloat16, kind="ExternalInput")
    w_d = nc.dram_tensor(
        "w", [128, NTAPS * 128], mybir.dt.bfloat16, kind="ExternalInput"
    )
    hot_d = nc.dram_tensor(
        "hot", [128, HOT_COLS], mybir.dt.bfloat16, kind="ExternalInput"
    )
    b_d = nc.dram_tensor("b", [128, 1], mybir.dt.float32, kind="ExternalInput")
    o_d = nc.dram_tensor(
        "out", [128, IMG_PER_CORE * QOUT], mybir.dt.bfloat16, kind="ExternalOutput"
    )

    ident = mybir.ActivationFunctionType.Identity

    chunk_plan = {}
    s = 0
    for n in CHUNKS:
        chunk_plan[s] = n
        s += n
    assert s == N_SLOTS
    stage_plan = {}
    s = 0
    for n in STAGES:
        stage_plan[s] = n
        s += n
    assert s == N_SLOTS

    with tile.TileContext(nc) as tc:
        with (
            tc.tile_pool(name="const", bufs=1) as cpool,
            tc.tile_pool(name="xin", bufs=4) as xpool,
            tc.tile_pool(name="stage", bufs=3) as spool,
            tc.tile_pool(name="psum", bufs=3, space="PSUM") as ppool,
        ):
            # Each DMA has a ~2.4us fixed issue->semaphore latency, so the
            # whole slot-0 working set (w taps 0-2 + x_pad rows 0-3) ships as
            # ONE early "hot" transfer; taps 0-2 are served from this
            # resident tile for every slot. Taps 3-8 follow in a second DMA
            # that lands before the cold stream reaches tap 3.
            ht = cpool.tile([128, HOT_COLS], mybir.dt.bfloat16, tag="hot")
            # split across both HWDGE rings so the ramping head bandwidth of
            # each ring works on half; region-level tile deps cover readers
            nc.sync.dma_start(ht[:, : HOT_COLS // 2], hot_d[:, : HOT_COLS // 2])
            nc.scalar.dma_start(ht[:, HOT_COLS // 2 :], hot_d[:, HOT_COLS // 2 :])
            w_sb = cpool.tile([128, NTAPS * 128], mybir.dt.bfloat16, tag="w")
            nc.sync.dma_start(w_sb[:, 384:1152], w_d[:, 384:1152])
            b_sb = cpool.tile([128, 1], mybir.dt.float32, tag="b")
            xv0 = ht[:, 384 : 384 + HOT_ROWS * PC].rearrange("p (r w) -> p r w", w=PC)

            # One HAM activity window (~3.4us) of dummy cold matmuls on a
            # zeroed tile, sized to finish as the first x chunk lands: the
            # PE clock-gate releases before the real stream starts, so it
            # runs at 2.4GHz early in the real stream (results never read).
            warm_src = cpool.tile([128, NSLOT], mybir.dt.bfloat16, tag="warm")
            nc.gpsimd.memset(warm_src[:], 0.0)
            warm_ps = ppool.tile([128, 512], mybir.dt.float32, tag="pswarm", bufs=1)
            for i in range(N_WARM):
                nc.tensor.matmul(
                    warm_ps[:, :NSLOT],
                    lhsT=warm_src[:, 0:128],
                    rhs=warm_src[:, :],
                    start=(i == 0),
                    stop=(i == N_WARM - 1),
                )

            xv = None
            ca = 0
            st_a = st_b = None
            g0 = 0
            gext = 0
            stage_end = -1
            for s in range(N_SLOTS):
                if s in chunk_plan:
                    ca = s
                    if s == 0:
                        xv = xv0  # slot 0 reads the resident hot tile
                        nc.sync.dma_start(b_sb[:], b_d[:])
                    else:
                        n_c = chunk_plan[s]
                        ext = (2 * n_c + 2) * PC
                        xt = xpool.tile([128, XTILE_COLS], mybir.dt.bfloat16, tag="x")
                        nc.sync.dma_start(
                            xt[:, :ext], x_d[:, 2 * ca * PC : 2 * ca * PC + ext]
                        )
                        xv = xt[:, :ext].rearrange("p (r w) -> p r w", w=PC)

                if s in stage_plan:
                    g0 = s * NSLOT
                    gext = stage_plan[s] * NSLOT
                    stage_end = s + stage_plan[s] - 1
                    st_a = spool.tile([128, STAGE_Q], mybir.dt.bfloat16, tag="sa")
                    st_b = spool.tile([128, STAGE_Q], mybir.dt.bfloat16, tag="sb")

                psa = ppool.tile([128, 512], mybir.dt.float32, tag="psa")
                psb = ppool.tile([128, 512], mybir.dt.float32, tag="psb")

                for t in range(NTAPS):
                    dh, dw = divmod(t, 3)
                    r0 = 2 * (s - ca) + dh
                    first, last = t == 0, t == NTAPS - 1
                    wt = ht if t < 3 else w_sb
                    nc.tensor.matmul(
                        psa[:, :NSLOT],
                        lhsT=wt[0:64, t * 128 : (t + 1) * 128],
                        rhs=xv[0:64, r0 : r0 + 2, dw : dw + W_IMG],
                        start=first,
                        stop=last,
                    )
                    nc.tensor.matmul(
                        psb[:, :NSLOT],
                        lhsT=wt[64:128, t * 128 : (t + 1) * 128],
                        rhs=xv[64:128, r0 : r0 + 2, dw : dw + W_IMG],
                        start=first,
                        stop=last,
                    )

                so = s * NSLOT - g0
                nc.vector.tensor_scalar_add(
                    st_a[:, so : so + NSLOT], psa[:, :NSLOT], b_sb[:]
                )
                nc.scalar.activation(
                    st_b[:, so : so + NSLOT], psb[:, :NSLOT], ident, bias=b_sb[:]
                )

                if s == stage_end:
                    nc.sync.dma_start(o_d[:, g0 : g0 + gext], st_a[:, :gext])
                    nc.scalar.dma_start(
                        o_d[:, QOUT + g0 : QOUT + g0 + gext], st_b[:, :gext]
                    )

    nc.compile()
    return nc


def _get_nc():
    global _COMPILED
    if _COMPILED is None:
        _COMPILED = _build()
    return _COMPILED


def kernel(x: np.ndarray, W: np.ndarray, b: np.ndarray) -> np.ndarray:
    from concourse.bass_utils import run_bass_kernel_spmd

    nc = _get_nc()

    xb = np.asarray(x, dtype=np.float32).astype(BF16)
    X = np.zeros((N_CORES, IMG_PER_CORE, C_IN, PR, PC), BF16)
    X[:, :, :, 1 : H + 1, 1 : W_IMG + 1] = xb.reshape(
        N_CORES, IMG_PER_CORE, H, W_IMG, C_IN
    ).transpose(0, 1, 4, 2, 3)
    Xf = X.reshape(N_CORES, 128, PLANE)

    Wb = np.sign(np.asarray(W, dtype=np.float32)).astype(BF16).reshape(NTAPS, C_IN, C_OUT)
    wh = np.empty((2, C_IN, NTAPS, C_OUT), BF16)
    wh[:] = Wb.transpose(1, 0, 2)[None]
    wh = np.ascontiguousarray(wh.reshape(128, NTAPS * C_OUT))

    bh = np.ascontiguousarray(np.asarray(b, dtype=np.float32).reshape(128, 1))

    hot = [
        np.ascontiguousarray(
            np.concatenate([wh[:, : 3 * 128], Xf[c][:, : HOT_ROWS * PC]], axis=1)
        )
        for c in range(N_CORES)
    ]

    in_maps = [
        {"x": Xf[c], "w": wh, "b": bh, "hot": hot[c]} for c in range(N_CORES)
    ]
    res = run_bass_kernel_spmd(nc, in_maps, list(range(N_CORES)))
    global _LAST_RES
    _LAST_RES = res

    O = np.stack([res.results[c]["out"] for c in range(N_CORES)])
    O = O.reshape(N_CORES, C_OUT, IMG_PER_CORE, H, W_IMG)
    y = O.transpose(0, 2, 3, 4, 1).reshape(16, H, W_IMG, C_OUT)
    return np.ascontiguousarray(y).astype(np.float32)
